# revision 18
# baseline (speedup 1.0000x reference)
"""CNF vector field + exact divergence kernel for Trainium2 (8 NeuronCores).

Math (per sample x of dim D=64, t scalar, 3-layer MLP 65->512->512->64):
    h1 = tanh(W1hat^T [x;t;1])       (w1hat rows: W1[1:], W1[0], b1)
    h2 = tanh(W2^T h1 + b2)
    dx = W3^T h2 + b3
    div = trace(d dx / d x) = (h1^2-1)^T G (h2^2-1)
    with G = W2 * (W1[1:].T @ W3.T)  computed on HOST (weights-only).

Layout: everything on device is feature-major ([feature partitions, batch
free]).  The host pre-transposes x into xhat^T = [x;t;1]^T (feature-major)
and transposes the [65, B] device output back to [B, 65] -- zero PE
transposes on device.

Per-core per-chunk (CH=512 batch columns, 2 chunks):
  L1  4 MMs  -> pz1 big PSUM [128,4,512] -> 1 big tanh -> h1
  DVE: sq1 = h1*h1 (big), m1 = sq1 - 1 (big)
  L2  16 MMs -> pz2 (per i-tile) -> tanh+bias b2 -> h2 (4 ACT)
  DVE: sq2 = h2*h2 (big), m2 = sq2 - 1 (big)
  c   16 MMs -> pc (per i-tile) -> e_i = m2_i * pc_i (DVE TT, PSUM read)
  div 4 ones-MMs accumulate pd[1,CH] over e k-tiles (no DVE tree)
  L3  4 MMs -> po -> ACT +b3 -> outT[0:64]; ACT pd -> outT[64]
  out DMA outT [65, CH] -> DRAM feature-major; host transposes back.

gpsimd does NOTHING (v1 measured 7.5us per elementwise op there).
"""

import sys

if "/opt/trn_rl_repo" not in sys.path:
    sys.path.insert(0, "/opt/trn_rl_repo")

import numpy as np

D = 64
H = 512
B = 8192
N_CORES = 8
BC = B // N_CORES          # 1024 samples per core
NCH = 2                    # batch chunks per core
CH = BC // NCH             # 512
KT = H // 128              # 4 k-tiles of the hidden dim

MM_DTYPE = "bfloat16"      # or "float32r"

_CACHE = {}


def _patch_tile_drain():
    """walrus in this toolchain accepts only one sync wait per CTRL
    instruction; split the TileContext tail-drain waits across nops."""
    import concourse.mybir as mybir
    from concourse.tile import TileContext
    from concourse.vector_clock import ScopedClock

    if getattr(TileContext, "_drain_patched", False):
        return

    def _drain_and_barrier(self, tick_clock, wait_clock):
        # Distribute the tail sem-waits across all engines (walrus accepts
        # only one wait per instruction, so serial SP nops cost ~3us) and
        # skip the cross-engine EVSEM barrier: each engine stream simply
        # ends once its waits are satisfied; NRT completion requires all
        # engine queues + DMA queues done, which the drains cover.
        nc = self.nc
        probe = nc.sync.nop(nofuse=True, hint="drain_wait_probe")
        wait_clock.add_sem_waits(
            probe.ins, ScopedClock({None: tick_clock.global_clock})
        )
        waits = list(probe.ins.sync_info.on_wait) if probe.ins.sync_info else []
        if probe.ins.sync_info is not None:
            probe.ins.sync_info.on_wait.clear()
        engines = [nc.sync, nc.scalar, nc.vector, nc.tensor, nc.gpsimd]
        for idx, w in enumerate(waits):
            eng = engines[idx % len(engines)]
            nop_inst = eng.nop(nofuse=True, hint=f"drain_wait_{idx}")
            if nop_inst.ins.sync_info is None:
                nop_inst.ins.sync_info = mybir.SyncInfo(on_wait=[], on_update=[])
            nop_inst.ins.sync_info.on_wait.append(w)
        nc.sync.drain()
        nc.scalar.drain()  # both DMA-issuing engines drain their queues
        popped = nc._tile_sem_poison_stack.pop()
        assert popped is self._sem_poison
        # sem clears skipped: NRT reloads sem state per execution; verified
        # by repeated-call correctness checks in test.py

    TileContext._orig_drain_and_barrier = TileContext._drain_and_barrier
    TileContext._drain_and_barrier = _drain_and_barrier
    TileContext._drain_patched = True


# this walrus build has small per-instruction sync-wait budgets; split any
# excess waits onto same-engine nops placed just before the instruction
# (waiting earlier on the same engine stream is always safe).
_WAIT_LIMITS = {"DMACOPY": 1, "NOOP": 1, "DRAIN": 1, "TRIGGEREDCOPY": 1}
_DEFAULT_WAIT_LIMIT = 1


def _split_excess_waits(nc):
    import concourse.mybir as mybir

    ctr = 0
    for fn in nc.m.functions:
        for blk in fn.blocks:
            lst = blk.instructions
            out = []
            changed = False
            for inst in lst:
                si = inst.sync_info
                waits = list(si.on_wait) if si else []
                opname = type(inst).__name__.replace("Inst", "").upper()
                limit = _WAIT_LIMITS.get(opname, _DEFAULT_WAIT_LIMIT)
                if len(waits) > limit:
                    keep = waits[-limit:]
                    excess = waits[:-limit]
                    si.on_wait.clear()
                    for w in keep:
                        si.on_wait.append(w)
                    for w in excess:
                        nop = mybir.InstNoOp(name=f"WSPLIT-{ctr}", ins=[], outs=[])
                        ctr += 1
                        nop.engine = inst.engine
                        nop.sync_info = mybir.SyncInfo(on_wait=[w], on_update=[])
                        out.append(nop)
                    changed = True
                out.append(inst)
            if changed:
                lst[:] = out


def _build(mm_dtype_name=MM_DTYPE, for_sim=False):
    import concourse.bass as bass
    import concourse.mybir as mybir
    from concourse.tile import TileContext

    _patch_tile_drain()

    f32 = mybir.dt.float32
    mmdt = getattr(mybir.dt, mm_dtype_name)
    AF = mybir.ActivationFunctionType

    f32r_like = mm_dtype_name in ("float32", "float32r")
    # dram dtype for matmul-operand tensors: f32 bits for f32r (bitcast
    # views), native mmdt (bf16) otherwise
    ddt = f32 if f32r_like else mmdt

    nc = bass.Bass(trn_type="TRN2")
    nc._bass_sim_build = for_sim

    # host-prepped inputs, all pre-packed into device layout so every DMA
    # moves large contiguous per-partition lines:
    #   xw_h  [66, BC + H]    = [xhat | w1hat]         (3KB lines bf16)
    #   w2p/gp [128, KT*H]    = rearranged (t p) m -> p (t m)  (4KB lines)
    #   w3p   [128, KT*D]
    xw_h = nc.dram_tensor("xw_h", [D + 2, BC + H], ddt, kind="ExternalInput")
    w2p_h = nc.dram_tensor("w2p_h", [128, KT * H], ddt, kind="ExternalInput")
    gp_h = nc.dram_tensor("gp_h", [128, KT * H], ddt, kind="ExternalInput")
    w3p_h = nc.dram_tensor("w3p_h", [128, KT * D], ddt, kind="ExternalInput")
    # bias pack (f32): col 0..KT-1 = b2 tiles, col KT = b3 (rows 0..63)
    bias_h = nc.dram_tensor("bias_h", [128, KT + 1], f32, kind="ExternalInput")
    ones_h = nc.dram_tensor("ones_h", [128, 1], ddt, kind="ExternalInput")
    out_f = nc.dram_tensor("out_f", [D + 1, BC], f32, kind="ExternalOutput")

    def dm(ap):
        # dram-side view for DMA into mmdt tiles (f32r is a bitcast of f32)
        return ap.bitcast(mmdt) if f32r_like else ap

    with TileContext(nc) as tc:
        with (
            tc.tile_pool(name="weights", bufs=1) as wpool,
            tc.tile_pool(name="acts", bufs=1) as apool,
            tc.tile_pool(name="psmm", bufs=7, space="PSUM") as psmm,
        ):
            # ---------------- input / weight DMAs --------------------------
            # sync queue carries the critical path in need-order; scalar
            # queue starts with a dummy tanh so walrus emits the ACT table
            # load immediately (instead of right before the first real tanh)
            dummy = wpool.tile([1, 1], f32)
            nc.scalar.activation(
                dummy, nc.const_aps.scalar_like(1.0, dummy), AF.Tanh
            )

            xw = apool.tile([D + 2, BC + H], mmdt)
            nc.sync.dma_start(out=xw, in_=dm(xw_h[:]))
            xh = xw[:, 0:BC]
            w1hat = xw[:, BC : BC + H]
            w24 = wpool.tile([128, KT, H], mmdt)
            nc.sync.dma_start(
                out=w24, in_=dm(w2p_h[:].rearrange("p (t m) -> p t m", t=KT))
            )
            ones_col = wpool.tile([128, 1], mmdt)
            nc.sync.dma_start(out=ones_col, in_=dm(ones_h[:]))
            bias_t = wpool.tile([128, KT + 1], f32)
            nc.scalar.dma_start(out=bias_t, in_=bias_h[:])
            b2t = bias_t[:, 0:KT]
            b3t = bias_t[0:D, KT : KT + 1]
            g24 = wpool.tile([128, KT, H], mmdt)
            nc.scalar.dma_start(
                out=g24, in_=dm(gp_h[:].rearrange("p (t m) -> p t m", t=KT))
            )
            w34 = wpool.tile([128, KT, D], mmdt)
            nc.scalar.dma_start(
                out=w34, in_=dm(w3p_h[:].rearrange("p (t m) -> p t m", t=KT))
            )

            # per-chunk activation tiles
            h1 = [apool.tile([128, KT, CH], mmdt, tag=f"h1_{n}", name=f"h1_{n}") for n in range(NCH)]
            m1 = [apool.tile([128, KT, CH], mmdt, tag=f"m1_{n}", name=f"m1_{n}") for n in range(NCH)]
            sq1 = [apool.tile([128, KT, CH], mmdt, tag=f"sq1_{n}", name=f"sq1_{n}") for n in range(NCH)]
            h2 = [apool.tile([128, KT, CH], mmdt, tag=f"h2_{n}", name=f"h2_{n}") for n in range(NCH)]
            m2 = [apool.tile([128, KT, CH], mmdt, tag=f"m2_{n}", name=f"m2_{n}") for n in range(NCH)]
            sq2 = [apool.tile([128, KT, CH], mmdt, tag=f"sq2_{n}", name=f"sq2_{n}") for n in range(NCH)]
            ebuf = [apool.tile([128, KT, CH], mmdt, tag=f"eb{n}", name=f"eb{n}") for n in range(NCH)]

            for n in range(NCH):
                xslice = xh[:, n * CH : (n + 1) * CH]

                # -------- L1 -----------------------------------------------
                for i in range(KT):
                    pz = psmm.tile([128, CH], f32, tag="mmtile")
                    nc.tensor.matmul(
                        pz,
                        w1hat[:, i * 128 : (i + 1) * 128],
                        xslice,
                        start=True,
                        stop=True,
                    )
                    nc.scalar.activation(h1[n][:, i, :], pz, AF.Tanh)
                nc.vector.tensor_mul(sq1[n][:], h1[n][:], h1[n][:])
                nc.vector.tensor_scalar_sub(m1[n][:], sq1[n][:], 1.0)

                # -------- L2: per-i-tile (bias b2 varies per tile) ---------
                # sq2/m2 per-i-tile so e_i unlocks as soon as tanh2_i lands
                for i in range(KT):
                    pz = psmm.tile([128, CH], f32, tag="mmtile")
                    for k in range(KT):
                        nc.tensor.matmul(
                            pz,
                            w24[:, k, i * 128 : (i + 1) * 128],
                            h1[n][:, k, :],
                            start=(k == 0),
                            stop=(k == KT - 1),
                        )
                    nc.scalar.activation(
                        h2[n][:, i, :], pz, AF.Tanh, bias=b2t[:, i : i + 1]
                    )
                    nc.vector.tensor_mul(
                        sq2[n][:, i, :], h2[n][:, i, :], h2[n][:, i, :]
                    )
                    nc.vector.tensor_scalar_sub(
                        m2[n][:, i, :], sq2[n][:, i, :], 1.0
                    )

                # -------- L3 (only needs h2; PE work while DVE does e) -----
                # dx rows DMA out early; the tiny dv row goes separately on
                # the scalar queue so the kernel tail is a 2KB transfer
                outT = apool.tile([D, CH], f32, tag=f"outT{n}", name=f"outT{n}")
                dvT = apool.tile([1, CH], f32, tag=f"dvT{n}", name=f"dvT{n}")
                po = psmm.tile([D, CH], f32, tag="mmtile", name=f"po{n}")
                for k in range(KT):
                    nc.tensor.matmul(
                        po,
                        w34[:, k, :],
                        h2[n][:, k, :],
                        start=(k == 0),
                        stop=(k == KT - 1),
                    )
                nc.scalar.activation(outT, po, AF.Identity, bias=b3t)
                nc.sync.dma_start(
                    out=out_f[0:D, n * CH : (n + 1) * CH], in_=outT
                )

                # -------- c = G^T m1 ; e_i = m2_i * pc_i -------------------
                for i in range(KT):
                    pc = psmm.tile([128, CH], f32, tag="mmtile")
                    for k in range(KT):
                        nc.tensor.matmul(
                            pc,
                            g24[:, k, i * 128 : (i + 1) * 128],
                            m1[n][:, k, :],
                            start=(k == 0),
                            stop=(k == KT - 1),
                        )
                    nc.vector.tensor_mul(ebuf[n][:, i, :], m2[n][:, i, :], pc)

                # -------- div: 4 accumulating ones-MMs over e k-tiles ------
                pd = psmm.tile([1, CH], f32, tag="mmtile", name=f"pd{n}")
                for k in range(KT):
                    nc.tensor.matmul(
                        pd,
                        ones_col,
                        ebuf[n][:, k, :],
                        start=(k == 0),
                        stop=(k == KT - 1),
                    )
                nc.scalar.activation(dvT, pd, AF.Identity)
                nc.scalar.dma_start(
                    out=out_f[D : D + 1, n * CH : (n + 1) * CH], in_=dvT
                )

    if not for_sim:
        _split_excess_waits(nc)
    return nc


def _get_nc():
    if "nc" not in _CACHE:
        _CACHE["nc"] = _build()
    return _CACHE["nc"]


def _np_ddt():
    import concourse.mybir as mybir

    if MM_DTYPE in ("float32", "float32r"):
        return np.float32
    return mybir.dt.np(getattr(mybir.dt, MM_DTYPE))


def _make_in_maps(inputs):
    t = np.asarray(inputs["t"], np.float32)
    x = np.asarray(inputs["x"], np.float32)
    W1 = np.asarray(inputs["W1"], np.float32)
    b1 = np.asarray(inputs["b1"], np.float32)
    W2 = np.asarray(inputs["W2"], np.float32)
    b2 = np.asarray(inputs["b2"], np.float32)
    W3 = np.asarray(inputs["W3"], np.float32)
    b3 = np.asarray(inputs["b3"], np.float32)
    ddt = _np_ddt()

    # feature-major xhat: rows 0..63 = x^T, row 64 = t, row 65 = 1
    xhat = np.empty((D + 2, B), np.float32)
    xhat[0:D] = x[:, 0:D].T
    xhat[D] = t[0]
    xhat[D + 1] = 1.0
    xhat = xhat.astype(ddt)

    w1hat = np.concatenate([W1[1:], W1[0:1], b1[None]], axis=0).astype(ddt)

    # host G = W2 * (W1[1:].T @ W3.T)   [H, H]
    G = (W2 * (W1[1:].T @ W3.T)).astype(np.float32)

    def pk(w):  # [H, M] -> [128, KT*M] device layout, big DMA lines
        m = w.shape[1]
        return np.ascontiguousarray(
            w.reshape(KT, 128, m).transpose(1, 0, 2).reshape(128, KT * m)
        ).astype(ddt)

    bias = np.zeros((128, KT + 1), np.float32)
    bias[:, 0:KT] = b2.reshape(KT, 128).T
    bias[0:D, KT] = b3

    base = {
        "w2p_h": pk(W2),
        "gp_h": pk(G),
        "w3p_h": pk(W3),
        "bias_h": bias,
        "ones_h": np.ones((128, 1), ddt),
    }
    maps = []
    for i in range(N_CORES):
        xw = np.empty((D + 2, BC + H), ddt)
        xw[:, 0:BC] = xhat[:, i * BC : (i + 1) * BC]
        xw[:, BC : BC + H] = w1hat
        maps.append(dict(base, xw_h=xw))
    return maps


def _gather(res):
    # device output is feature-major [65, BC]; transpose back per core
    return np.concatenate(
        [np.ascontiguousarray(res.results[i]["out_f"].T) for i in range(N_CORES)],
        axis=0,
    )


def kernel(t, x, W1, b1, W2, b2, W3, b3):
    from concourse.bass_utils import run_bass_kernel_spmd

    nc = _get_nc()
    in_maps = _make_in_maps(
        dict(t=t, x=x, W1=W1, b1=b1, W2=W2, b2=b2, W3=W3, b3=b3)
    )
    res = run_bass_kernel_spmd(nc, in_maps, core_ids=list(range(N_CORES)))
    _CACHE["last_result"] = res
    out = _gather(res)
    # flaky-core guard: a dropped execution leaves the donated zero output
    # buffer untouched; the true output of this MLP is never all-zero.
    for _ in range(3):
        bad = [
            i
            for i in range(N_CORES)
            if not np.any(res.results[i]["out_f"][0:D, :])
        ]
        if not bad:
            break
        res = run_bass_kernel_spmd(nc, in_maps, core_ids=list(range(N_CORES)))
        _CACHE["last_result"] = res
        out = _gather(res)
    return out


# revision 19
# speedup vs baseline: 1.2960x; 1.2960x over previous
"""CNF vector field + exact divergence kernel for Trainium2 (8 NeuronCores).

Math (per sample x of dim D=64, t scalar, 3-layer MLP 65->512->512->64):
    h1 = tanh(W1hat^T [x;t;1])       (w1hat rows: W1[1:], W1[0], b1)
    h2 = tanh(W2^T h1 + b2)
    dx = W3^T h2 + b3
    div = trace(d dx / d x) = (h1^2-1)^T G (h2^2-1)
    with G = W2 * (W1[1:].T @ W3.T)  computed on HOST (weights-only).

Layout: everything on device is feature-major ([feature partitions, batch
free]).  The host pre-transposes x into xhat^T = [x;t;1]^T (feature-major)
and transposes the [65, B] device output back to [B, 65] -- zero PE
transposes on device.

Per-core per-chunk (CH=512 batch columns, 2 chunks):
  L1  4 MMs  -> pz1 big PSUM [128,4,512] -> 1 big tanh -> h1
  DVE: sq1 = h1*h1 (big), m1 = sq1 - 1 (big)
  L2  16 MMs -> pz2 (per i-tile) -> tanh+bias b2 -> h2 (4 ACT)
  DVE: sq2 = h2*h2 (big), m2 = sq2 - 1 (big)
  c   16 MMs -> pc (per i-tile) -> e_i = m2_i * pc_i (DVE TT, PSUM read)
  div 4 ones-MMs accumulate pd[1,CH] over e k-tiles (no DVE tree)
  L3  4 MMs -> po -> ACT +b3 -> outT[0:64]; ACT pd -> outT[64]
  out DMA outT [65, CH] -> DRAM feature-major; host transposes back.

gpsimd does NOTHING (v1 measured 7.5us per elementwise op there).
"""

import sys

if "/opt/trn_rl_repo" not in sys.path:
    sys.path.insert(0, "/opt/trn_rl_repo")

import numpy as np

D = 64
H = 512
B = 8192
N_CORES = 8
BC = B // N_CORES          # 1024 samples per core
NCH = 2                    # batch chunks per core
CH = BC // NCH             # 512
KT = H // 128              # 4 k-tiles of the hidden dim

MM_DTYPE = "bfloat16"      # or "float32r"

_CACHE = {}


def _patch_tile_drain():
    """walrus in this toolchain accepts only one sync wait per CTRL
    instruction; split the TileContext tail-drain waits across nops."""
    import concourse.mybir as mybir
    from concourse.tile import TileContext
    from concourse.vector_clock import ScopedClock

    if getattr(TileContext, "_drain_patched", False):
        return

    def _drain_and_barrier(self, tick_clock, wait_clock):
        # Distribute the tail sem-waits across all engines (walrus accepts
        # only one wait per instruction, so serial SP nops cost ~3us) and
        # skip the cross-engine EVSEM barrier: each engine stream simply
        # ends once its waits are satisfied; NRT completion requires all
        # engine queues + DMA queues done, which the drains cover.
        nc = self.nc
        probe = nc.sync.nop(nofuse=True, hint="drain_wait_probe")
        wait_clock.add_sem_waits(
            probe.ins, ScopedClock({None: tick_clock.global_clock})
        )
        waits = list(probe.ins.sync_info.on_wait) if probe.ins.sync_info else []
        if probe.ins.sync_info is not None:
            probe.ins.sync_info.on_wait.clear()
        engines = [nc.sync, nc.scalar, nc.vector, nc.tensor, nc.gpsimd]
        for idx, w in enumerate(waits):
            eng = engines[idx % len(engines)]
            nop_inst = eng.nop(nofuse=True, hint=f"drain_wait_{idx}")
            if nop_inst.ins.sync_info is None:
                nop_inst.ins.sync_info = mybir.SyncInfo(on_wait=[], on_update=[])
            nop_inst.ins.sync_info.on_wait.append(w)
        nc.sync.drain()
        nc.scalar.drain()  # both DMA-issuing engines drain their queues
        popped = nc._tile_sem_poison_stack.pop()
        assert popped is self._sem_poison
        # sem clears skipped: NRT reloads sem state per execution; verified
        # by repeated-call correctness checks in test.py

    TileContext._orig_drain_and_barrier = TileContext._drain_and_barrier
    TileContext._drain_and_barrier = _drain_and_barrier
    TileContext._drain_patched = True


# this walrus build has small per-instruction sync-wait budgets; split any
# excess waits onto same-engine nops placed just before the instruction
# (waiting earlier on the same engine stream is always safe).
_WAIT_LIMITS = {"DMACOPY": 1, "NOOP": 1, "DRAIN": 1, "TRIGGEREDCOPY": 1}
_DEFAULT_WAIT_LIMIT = 1


def _split_excess_waits(nc):
    import concourse.mybir as mybir

    ctr = 0
    for fn in nc.m.functions:
        for blk in fn.blocks:
            lst = blk.instructions
            out = []
            changed = False
            for inst in lst:
                si = inst.sync_info
                waits = list(si.on_wait) if si else []
                opname = type(inst).__name__.replace("Inst", "").upper()
                limit = _WAIT_LIMITS.get(opname, _DEFAULT_WAIT_LIMIT)
                if len(waits) > limit:
                    keep = waits[-limit:]
                    excess = waits[:-limit]
                    si.on_wait.clear()
                    for w in keep:
                        si.on_wait.append(w)
                    for w in excess:
                        nop = mybir.InstNoOp(name=f"WSPLIT-{ctr}", ins=[], outs=[])
                        ctr += 1
                        nop.engine = inst.engine
                        nop.sync_info = mybir.SyncInfo(on_wait=[w], on_update=[])
                        out.append(nop)
                    changed = True
                out.append(inst)
            if changed:
                lst[:] = out


def _build(mm_dtype_name=MM_DTYPE, for_sim=False):
    import concourse.bass as bass
    import concourse.mybir as mybir
    from concourse.tile import TileContext

    _patch_tile_drain()

    f32 = mybir.dt.float32
    mmdt = getattr(mybir.dt, mm_dtype_name)
    AF = mybir.ActivationFunctionType

    f32r_like = mm_dtype_name in ("float32", "float32r")
    # dram dtype for matmul-operand tensors: f32 bits for f32r (bitcast
    # views), native mmdt (bf16) otherwise
    ddt = f32 if f32r_like else mmdt

    nc = bass.Bass(trn_type="TRN2")
    nc._bass_sim_build = for_sim

    # host-prepped inputs, all pre-packed into device layout so every DMA
    # moves large contiguous per-partition lines:
    #   xw_h  [66, BC + H]    = [xhat | w1hat]         (3KB lines bf16)
    #   w2p/gp [128, KT*H]    = rearranged (t p) m -> p (t m)  (4KB lines)
    #   w3p   [128, KT*D]
    xw_h = nc.dram_tensor("xw_h", [D + 2, BC + H], ddt, kind="ExternalInput")
    w2p_h = nc.dram_tensor("w2p_h", [128, KT * H], ddt, kind="ExternalInput")
    gp_h = nc.dram_tensor("gp_h", [128, KT * H], ddt, kind="ExternalInput")
    w3p_h = nc.dram_tensor("w3p_h", [128, KT * D], ddt, kind="ExternalInput")
    # bias pack (f32): col 0..KT-1 = b2 tiles, col KT = b3 (rows 0..63)
    bias_h = nc.dram_tensor("bias_h", [128, KT + 1], f32, kind="ExternalInput")
    ones_h = nc.dram_tensor("ones_h", [128, 1], ddt, kind="ExternalInput")
    out_f = nc.dram_tensor("out_f", [D + 1, BC], f32, kind="ExternalOutput")

    def dm(ap):
        # dram-side view for DMA into mmdt tiles (f32r is a bitcast of f32)
        return ap.bitcast(mmdt) if f32r_like else ap

    with TileContext(nc) as tc:
        with (
            tc.tile_pool(name="weights", bufs=1) as wpool,
            tc.tile_pool(name="acts", bufs=1) as apool,
            tc.tile_pool(name="psmm", bufs=7, space="PSUM") as psmm,
        ):
            # ---------------- input / weight DMAs --------------------------
            # sync queue carries the critical path in need-order; scalar
            # queue starts with a dummy tanh so walrus emits the ACT table
            # load immediately (instead of right before the first real tanh)
            dummy = wpool.tile([1, 1], f32)
            nc.scalar.activation(
                dummy, nc.const_aps.scalar_like(1.0, dummy), AF.Tanh
            )

            xw = apool.tile([D + 2, BC + H], mmdt)
            nc.sync.dma_start(out=xw, in_=dm(xw_h[:]))
            xh = xw[:, 0:BC]
            w1hat = xw[:, BC : BC + H]
            w24 = wpool.tile([128, KT, H], mmdt)
            nc.sync.dma_start(
                out=w24, in_=dm(w2p_h[:].rearrange("p (t m) -> p t m", t=KT))
            )
            ones_col = wpool.tile([128, 1], mmdt)
            nc.sync.dma_start(out=ones_col, in_=dm(ones_h[:]))
            bias_t = wpool.tile([128, KT + 1], f32)
            nc.scalar.dma_start(out=bias_t, in_=bias_h[:])
            b2t = bias_t[:, 0:KT]
            b3t = bias_t[0:D, KT : KT + 1]
            g24 = wpool.tile([128, KT, H], mmdt)
            nc.scalar.dma_start(
                out=g24, in_=dm(gp_h[:].rearrange("p (t m) -> p t m", t=KT))
            )
            w34 = wpool.tile([128, KT, D], mmdt)
            nc.scalar.dma_start(
                out=w34, in_=dm(w3p_h[:].rearrange("p (t m) -> p t m", t=KT))
            )

            # per-chunk activation tiles
            h1 = [apool.tile([128, KT, CH], mmdt, tag=f"h1_{n}", name=f"h1_{n}") for n in range(NCH)]
            m1 = [apool.tile([128, KT, CH], mmdt, tag=f"m1_{n}", name=f"m1_{n}") for n in range(NCH)]
            sq1 = [apool.tile([128, KT, CH], mmdt, tag=f"sq1_{n}", name=f"sq1_{n}") for n in range(NCH)]
            h2 = [apool.tile([128, KT, CH], mmdt, tag=f"h2_{n}", name=f"h2_{n}") for n in range(NCH)]
            m2 = [apool.tile([128, KT, CH], mmdt, tag=f"m2_{n}", name=f"m2_{n}") for n in range(NCH)]
            sq2 = [apool.tile([128, KT, CH], mmdt, tag=f"sq2_{n}", name=f"sq2_{n}") for n in range(NCH)]
            ebuf = [apool.tile([128, KT, CH], mmdt, tag=f"eb{n}", name=f"eb{n}") for n in range(NCH)]

            # -------- PE warm-up: dummy MMs on a memset tile ----------------
            # HAM un-throttles after ~3.4us of sustained PE activity; these
            # run while the input DMAs land so the real MMs start at 2.4GHz
            warm = wpool.tile([128, 512], mmdt)
            nc.vector.memset(warm, 1.0)
            pwarm = psmm.tile([128, CH], f32, tag="mmtile", name="pwarm")
            for _ in range(8):
                nc.tensor.matmul(
                    pwarm, warm[:, 0:128], warm, start=True, stop=True
                )

            def stage_l1(n):
                xslice = xh[:, n * CH : (n + 1) * CH]
                for i in range(KT):
                    pz = psmm.tile([128, CH], f32, tag="mmtile")
                    nc.tensor.matmul(
                        pz,
                        w1hat[:, i * 128 : (i + 1) * 128],
                        xslice,
                        start=True,
                        stop=True,
                    )
                    nc.scalar.activation(h1[n][:, i, :], pz, AF.Tanh)
                nc.vector.tensor_mul(sq1[n][:], h1[n][:], h1[n][:])
                nc.vector.tensor_scalar_sub(m1[n][:], sq1[n][:], 1.0)

            def stage_l2(n):
                # sq2/m2 per-i-tile so e_i unlocks as soon as tanh2_i lands
                for i in range(KT):
                    pz = psmm.tile([128, CH], f32, tag="mmtile")
                    for k in range(KT):
                        nc.tensor.matmul(
                            pz,
                            w24[:, k, i * 128 : (i + 1) * 128],
                            h1[n][:, k, :],
                            start=(k == 0),
                            stop=(k == KT - 1),
                        )
                    nc.scalar.activation(
                        h2[n][:, i, :], pz, AF.Tanh, bias=b2t[:, i : i + 1]
                    )
                    nc.vector.tensor_mul(
                        sq2[n][:, i, :], h2[n][:, i, :], h2[n][:, i, :]
                    )
                    nc.vector.tensor_scalar_sub(
                        m2[n][:, i, :], sq2[n][:, i, :], 1.0
                    )

            outTs = [
                apool.tile([D, CH], f32, tag=f"outT{n}", name=f"outT{n}")
                for n in range(NCH)
            ]
            dvTs = [
                apool.tile([1, CH], f32, tag=f"dvT{n}", name=f"dvT{n}")
                for n in range(NCH)
            ]

            def stage_l3(n):
                po = psmm.tile([D, CH], f32, tag="mmtile", name=f"po{n}")
                for k in range(KT):
                    nc.tensor.matmul(
                        po,
                        w34[:, k, :],
                        h2[n][:, k, :],
                        start=(k == 0),
                        stop=(k == KT - 1),
                    )
                nc.scalar.activation(outTs[n], po, AF.Identity, bias=b3t)
                nc.sync.dma_start(
                    out=out_f[0:D, n * CH : (n + 1) * CH], in_=outTs[n]
                )

            def stage_c(n):
                for i in range(KT):
                    pc = psmm.tile([128, CH], f32, tag="mmtile")
                    for k in range(KT):
                        nc.tensor.matmul(
                            pc,
                            g24[:, k, i * 128 : (i + 1) * 128],
                            m1[n][:, k, :],
                            start=(k == 0),
                            stop=(k == KT - 1),
                        )
                    nc.vector.tensor_mul(ebuf[n][:, i, :], m2[n][:, i, :], pc)

            def stage_div(n):
                pd = psmm.tile([1, CH], f32, tag="mmtile", name=f"pd{n}")
                for k in range(KT):
                    nc.tensor.matmul(
                        pd,
                        ones_col,
                        ebuf[n][:, k, :],
                        start=(k == 0),
                        stop=(k == KT - 1),
                    )
                nc.vector.tensor_copy(dvTs[n], pd)
                nc.scalar.dma_start(
                    out=out_f[D : D + 1, n * CH : (n + 1) * CH], in_=dvTs[n]
                )

            # interleave chunks: chunk1's L1 fills the PE bubble while ACT
            # runs chunk0's tanh; L3 runs while DVE computes e products
            stage_l1(0)
            stage_l1(1)
            stage_l2(0)
            stage_c(0)
            stage_l2(1)
            stage_l3(0)
            stage_div(0)
            stage_c(1)
            stage_l3(1)
            stage_div(1)

    if not for_sim:
        _split_excess_waits(nc)
    return nc


def _get_nc():
    if "nc" not in _CACHE:
        _CACHE["nc"] = _build()
    return _CACHE["nc"]


def _np_ddt():
    import concourse.mybir as mybir

    if MM_DTYPE in ("float32", "float32r"):
        return np.float32
    return mybir.dt.np(getattr(mybir.dt, MM_DTYPE))


def _make_in_maps(inputs):
    t = np.asarray(inputs["t"], np.float32)
    x = np.asarray(inputs["x"], np.float32)
    W1 = np.asarray(inputs["W1"], np.float32)
    b1 = np.asarray(inputs["b1"], np.float32)
    W2 = np.asarray(inputs["W2"], np.float32)
    b2 = np.asarray(inputs["b2"], np.float32)
    W3 = np.asarray(inputs["W3"], np.float32)
    b3 = np.asarray(inputs["b3"], np.float32)
    ddt = _np_ddt()

    # feature-major xhat: rows 0..63 = x^T, row 64 = t, row 65 = 1
    xhat = np.empty((D + 2, B), np.float32)
    xhat[0:D] = x[:, 0:D].T
    xhat[D] = t[0]
    xhat[D + 1] = 1.0
    xhat = xhat.astype(ddt)

    w1hat = np.concatenate([W1[1:], W1[0:1], b1[None]], axis=0).astype(ddt)

    # host G = W2 * (W1[1:].T @ W3.T)   [H, H]
    G = (W2 * (W1[1:].T @ W3.T)).astype(np.float32)

    def pk(w):  # [H, M] -> [128, KT*M] device layout, big DMA lines
        m = w.shape[1]
        return np.ascontiguousarray(
            w.reshape(KT, 128, m).transpose(1, 0, 2).reshape(128, KT * m)
        ).astype(ddt)

    bias = np.zeros((128, KT + 1), np.float32)
    bias[:, 0:KT] = b2.reshape(KT, 128).T
    bias[0:D, KT] = b3

    base = {
        "w2p_h": pk(W2),
        "gp_h": pk(G),
        "w3p_h": pk(W3),
        "bias_h": bias,
        "ones_h": np.ones((128, 1), ddt),
    }
    maps = []
    for i in range(N_CORES):
        xw = np.empty((D + 2, BC + H), ddt)
        xw[:, 0:BC] = xhat[:, i * BC : (i + 1) * BC]
        xw[:, BC : BC + H] = w1hat
        maps.append(dict(base, xw_h=xw))
    return maps


def _gather(res):
    # device output is feature-major [65, BC]; transpose back per core
    return np.concatenate(
        [np.ascontiguousarray(res.results[i]["out_f"].T) for i in range(N_CORES)],
        axis=0,
    )


def kernel(t, x, W1, b1, W2, b2, W3, b3):
    from concourse.bass_utils import run_bass_kernel_spmd

    nc = _get_nc()
    in_maps = _make_in_maps(
        dict(t=t, x=x, W1=W1, b1=b1, W2=W2, b2=b2, W3=W3, b3=b3)
    )
    res = run_bass_kernel_spmd(nc, in_maps, core_ids=list(range(N_CORES)))
    _CACHE["last_result"] = res
    out = _gather(res)
    # flaky-core guard: a dropped execution leaves the donated zero output
    # buffer untouched; the true output of this MLP is never all-zero.
    for _ in range(3):
        bad = [
            i
            for i in range(N_CORES)
            if not np.any(res.results[i]["out_f"][0:D, :])
        ]
        if not bad:
            break
        res = run_bass_kernel_spmd(nc, in_maps, core_ids=list(range(N_CORES)))
        _CACHE["last_result"] = res
        out = _gather(res)
    return out
